# revision 47
# baseline (speedup 1.0000x reference)
"""Trainium2 Bass kernel for nn_BaseGenerator (4-layer dense transformer).

Strategy: pure data-parallel over batch (B=8 -> 8 NeuronCores, no
collectives).  Each core runs the full transformer on one batch element.
Activations are kept feature-major [E, S] in bf16 so every GEMM contracts
over the partition dim; PSUM accumulates in fp32.

Host-side prep (layout only + the distance-bias embedding gather, which has
no efficient device path at 64B/row):
  - weights transposed/blocked into lhsT layouts, cast to bf16
  - attention additive mask  maskT[h, k, q] = dist_emb[dist[q,k], h]
    (+ -1e9 for causal / key-padding), bf16
  - the 1/sqrt(dh) score scale is folded into Wq/bq
"""

import os
import sys

for _p in ("/opt/trn_rl_repo",):
    if _p not in sys.path:
        sys.path.insert(0, _p)

import ml_dtypes
import numpy as np

import concourse.bass as bass
import concourse.mybir as mybir
import concourse.tile as tile
from concourse import bacc
from concourse.bass_utils import run_bass_kernel_spmd

BF16 = ml_dtypes.bfloat16

L, E, H, F = 4, 1024, 16, 4096
B, S = 8, 512
VV, VR = 40, 30
DIST_V = 200
PAD_ID = 0
DH = E // H  # 64
NE = E // 128  # 8 feature chunks
NO = 10  # logit row tiles (1280 padded)
NEG = -1.0e9

f32 = mybir.dt.float32
bf16 = mybir.dt.bfloat16
AF = mybir.ActivationFunctionType
OP = mybir.AluOpType

_CACHE = {}


# ----------------------------------------------------------------------------
# host-side input prep
# ----------------------------------------------------------------------------

def _prep_shared(inp):
    """Weight-layout prep shared by all cores. Returns dict name->np array."""
    out = {}

    def b16(x):
        return np.ascontiguousarray(x.astype(BF16))

    Wqkv = np.asarray(inp["Wqkv"], np.float32).copy()  # [L, 3E, E]
    bqkv = np.asarray(inp["bqkv"], np.float32).copy()  # [L, 3E]
    W1 = np.asarray(inp["W1"], np.float32).copy()      # [L, F, E]
    b1 = np.asarray(inp["b1"], np.float32).copy()      # [L, F]
    genW = np.asarray(inp["gen_W"], np.float32).copy()  # [1200, E]
    gen_b = np.asarray(inp["gen_b"], np.float32).copy()
    ln1_s = np.asarray(inp["ln1_s"], np.float32)
    ln1_b = np.asarray(inp["ln1_b"], np.float32)
    ln2_s = np.asarray(inp["ln2_s"], np.float32)
    ln2_b = np.asarray(inp["ln2_b"], np.float32)
    lnf_s = np.asarray(inp["lnf_s"], np.float32)
    lnf_b = np.asarray(inp["lnf_b"], np.float32)

    # Fold LN affine (per-feature scale/bias) into the consumers:
    #   h_real = t*s + b  with t the raw (x-mean)*rstd tiles kept on device.
    #   QKV/V of layer l>=1 consume ln2 of layer l-1; FFN1 consumes ln1;
    #   the head consumes the final LN.  Residual adds re-apply s via a
    #   fused scalar_tensor_tensor, with b folded into bo/b2.
    for l in range(1, L):
        bqkv[l] += Wqkv[l] @ ln2_b[l - 1]
        Wqkv[l] = Wqkv[l] * ln2_s[l - 1][None, :]
    for l in range(L):
        b1[l] += W1[l] @ ln1_b[l]
        W1[l] = W1[l] * ln1_s[l][None, :]
    gen_b = gen_b + genW @ lnf_b
    genW = genW * lnf_s[None, :]

    # Fold the V bias through attention into the out-proj bias: softmax
    # weights sum to 1, so a V bias adds the constant bv to every ctx vector
    # -> bo += Wo @ bv.  The V GEMM then needs no bias pass at all.
    Wo_raw = np.asarray(inp["Wo"], np.float32)
    bo = np.asarray(inp["bo"], np.float32).copy()
    for l in range(L):
        bo[l] += Wo_raw[l] @ bqkv[l, 2 * E:]

    # fold attention scale into Q projection (after the LN fold)
    scale = 1.0 / np.sqrt(DH)
    Wqkv[:, :E, :] *= scale
    bqkv[:, :E] *= scale

    def block_lhsT(W, gsize):
        # W: [L?, OUT, IN] -> [.., G, 128, IN//128, gsize] with
        # out[..., g, p, c, o] = W[..., g*gsize + o, c*128 + p]
        *lead, O, I = W.shape
        G = O // gsize
        nc_ = I // 128
        Wb = W.reshape(*lead, G, gsize, nc_, 128)
        Wb = np.moveaxis(Wb, -1, -3)  # [..., G, 128, gsize, nc]
        Wb = np.swapaxes(Wb, -1, -2)  # [..., G, 128, nc, gsize]
        return np.ascontiguousarray(Wb)

    out["wqkv"] = b16(block_lhsT(Wqkv, 512))          # [L, 6, 128, 8, 512]
    # Wo for packed ctx (2 heads per 128-partition tile):
    # wo[l, og, p, j, o] = Wo[l][og*256 + o, j*128 + p]
    Wo = np.asarray(inp["Wo"], np.float32)  # [L, E(out), E(in=ctx)]
    out["wo"] = b16(block_lhsT(Wo, 256))    # [L, 4, 128, 8, 256]
    out["w1"] = b16(block_lhsT(W1, 512))    # [L,8,128,8,512]
    # W2: [L, E, F]; stream tiles [half, cg, 128, 8, 512]:
    W2 = np.asarray(inp["W2"], np.float32)  # out=E, in=F
    w2b = block_lhsT(W2, 512)  # [L, 2, 128, 32, 512]
    w2b = w2b.reshape(L, 2, 128, 4, 8, 512).transpose(0, 1, 3, 2, 4, 5)
    out["w2"] = b16(w2b)  # [L, 2, 4, 128, 8, 512]

    genW_pad = np.zeros((1280, E), np.float32)
    genW_pad[:1200] = genW
    out["genw"] = b16(block_lhsT(genW_pad, 640))  # [2, 128, 8, 640]

    gbp = np.zeros((1280,), np.float32)
    gbp[:1200] = gen_b
    out["gen_b_pp"] = np.ascontiguousarray(gbp.reshape(NO, 128).T)  # [128, 10]

    def pp(v):  # [..., N*128] -> [..., 128, N]
        *lead, N = v.shape
        return np.ascontiguousarray(
            v.reshape(*lead, N // 128, 128).swapaxes(-1, -2).astype(np.float32)
        )

    out["bqkv_pp"] = pp(bqkv[:, : 2 * E])  # [L, 128, 16] (Q scaled)
    # residual-path biases with the producing-LN bias folded in
    bo[1:] += ln2_b[:L - 1]
    out["bo_pp"] = pp(bo)  # [L, 128, 8]
    out["b1_pp"] = pp(b1)  # [L, 128, 32]
    b2 = np.asarray(inp["b2"], np.float32) + ln1_b
    out["b2_pp"] = pp(b2)  # [L, 128, 8]

    ln_s = np.stack([ln1_s, ln2_s], 1)  # [L, 2, E]
    ln_b = np.stack([ln1_b, ln2_b], 1)
    out["ln_s_pp"] = pp(ln_s)  # [L, 2, 128, 8]
    out["ln_b_pp"] = pp(ln_b)
    out["lnf_s_pp"] = pp(lnf_s)  # [128, 8]
    out["lnf_b_pp"] = pp(lnf_b)

    out["valemb"] = b16(np.asarray(inp["val_emb"], np.float32))   # [40, E]
    out["ringemb"] = b16(np.asarray(inp["ring_emb"], np.float32))  # [30, E]

    out["id128"] = b16(np.eye(128, dtype=np.float32))
    out["ones_row"] = b16(np.ones((1, S), np.float32))
    out["iota_col"] = np.ascontiguousarray(np.arange(128, dtype=np.float32).reshape(128, 1))
    out["ones_col"] = b16(np.ones((128, 1), np.float32))
    return out


MW = (512, 384, 256, 128)          # emask widths per kc chunk (q >= kc*128)
MOFF = (0, 512, 896, 1152)         # col offsets of each chunk in the packed mask
MTOT = 1280


def _prep_percore(inp):
    """Per-core tensors: token rows + multiplicative exp-mask (triangular).

    emask[b, h, p, MOFF[kc]+j] = exp(bias) for key k = kc*128+p and query
    q = kc*128+j (0 where masked by causality / key padding)."""
    val = np.asarray(inp["val_sequences"]).astype(np.int64)    # [B, S]
    ring = np.asarray(inp["ring_sequences"]).astype(np.int64)  # [B, S]
    dist = np.asarray(inp["distance_squares"]).astype(np.int64)  # [B, S, S]
    de = np.asarray(inp["dist_emb"], np.float32)  # [200, H]

    # m[b, h, k, q] = exp(de[dist[b, q, k], h]), zeroed where masked
    m = de[dist]                         # [B, S(q), S(k), H]
    m = m.transpose(0, 3, 2, 1)          # [B, H, k, q]
    m = np.exp(m)
    kk = np.arange(S)
    causal = kk[:, None] <= kk[None, :]  # [k, q] keep where k <= q
    m = np.where(causal[None, None], m, 0.0)
    padk = val == PAD_ID  # [B, S]
    m = np.where(padk[:, None, :, None], 0.0, m)
    # pack triangular: chunk kc covers keys [kc*128,(kc+1)*128), q >= kc*128
    em = np.zeros((B, H, 128, MTOT), np.float32)
    for kc in range(4):
        em[:, :, :, MOFF[kc]:MOFF[kc] + MW[kc]] = \
            m[:, :, kc * 128:(kc + 1) * 128, kc * 128:]
    em = em.astype(BF16)

    cores = []
    for b in range(B):
        cores.append({
            "mask": np.ascontiguousarray(em[b]),
            "valrow": np.ascontiguousarray(val[b].reshape(1, S).astype(BF16)),
            "ringrow": np.ascontiguousarray(ring[b].reshape(1, S).astype(BF16)),
        })
    return cores


# ----------------------------------------------------------------------------
# device program
# ----------------------------------------------------------------------------

def _declare(nc):
    d = {}

    def di(name, shape, dt):
        d[name] = nc.dram_tensor(name, list(shape), dt, kind="ExternalInput").ap()

    di("wqkv", (L, 6, 128, 8, 512), bf16)
    di("wo", (L, 4, 128, 8, 256), bf16)
    di("w1", (L, 8, 128, 8, 512), bf16)
    di("w2", (L, 2, 4, 128, 8, 512), bf16)
    di("genw", (2, 128, 8, 640), bf16)
    di("gen_b_pp", (128, NO), f32)
    di("bqkv_pp", (L, 128, 16), f32)
    di("bo_pp", (L, 128, 8), f32)
    di("b1_pp", (L, 128, 32), f32)
    di("b2_pp", (L, 128, 8), f32)
    di("ln_s_pp", (L, 2, 128, 8), f32)
    di("ln_b_pp", (L, 2, 128, 8), f32)
    di("lnf_s_pp", (128, 8), f32)
    di("lnf_b_pp", (128, 8), f32)
    di("valemb", (VV, E), bf16)
    di("ringemb", (VR, E), bf16)
    di("id128", (128, 128), bf16)
    di("ones_row", (1, S), bf16)
    di("iota_col", (128, 1), f32)
    di("ones_col", (128, 1), bf16)
    di("mask", (H, 128, MTOT), bf16)
    di("valrow", (1, S), bf16)
    di("ringrow", (1, S), bf16)
    d["logits"] = nc.dram_tensor(
        "logits", [NO, 128, S], f32, kind="ExternalOutput"
    ).ap()
    if os.environ.get("BG_DEBUG"):
        def do(name, shape):
            d[name] = nc.dram_tensor(name, list(shape), bf16,
                                     kind="ExternalOutput").ap()
        do("dbg_h0", (NE, 128, S))
        do("dbg_qk", (16, 128, S))
        do("dbg_v", (4, 128, H, DH + 1))
        do("dbg_at", (8, 128, S))
        do("dbg_ctx", (H, DH, S))
        do("dbg_r1", (NE, 128, S))
        do("dbg_h1", (NE, 128, S))
        do("dbg_h2", (NE, 128, S))
    return d


def _emit(nc, tc, d, ctx):
    mm = nc.tensor.matmul

    cpool = ctx.enter_context(tc.tile_pool(name="cpool", bufs=1))
    wpool = ctx.enter_context(tc.tile_pool(name="wpool", bufs=3))
    hpool = ctx.enter_context(tc.tile_pool(name="hpool", bufs=17))
    qkpool = ctx.enter_context(tc.tile_pool(name="qkpool", bufs=16))
    vpool = ctx.enter_context(tc.tile_pool(name="vpool", bufs=5))
    maskpool = ctx.enter_context(tc.tile_pool(name="maskpool", bufs=4))
    atpool = ctx.enter_context(tc.tile_pool(name="atpool", bufs=10))
    ctxpool = ctx.enter_context(tc.tile_pool(name="ctxpool", bufs=10))
    ffpool = ctx.enter_context(tc.tile_pool(name="ffpool", bufs=33))
    tmppool = ctx.enter_context(tc.tile_pool(name="tmppool", bufs=4))
    smallf = ctx.enter_context(tc.tile_pool(name="smallf", bufs=4))
    smallb = ctx.enter_context(tc.tile_pool(name="smallb", bufs=6))
    recpool = ctx.enter_context(tc.tile_pool(name="recpool", bufs=2))
    outpool = ctx.enter_context(tc.tile_pool(name="outpool", bufs=2))
    pppool = ctx.enter_context(tc.tile_pool(name="pppool", bufs=4))

    ps_gemm = ctx.enter_context(tc.tile_pool(name="ps_gemm", bufs=4, space="PSUM"))
    ps_ctx = ctx.enter_context(tc.tile_pool(name="ps_ctx", bufs=3, space="PSUM"))

    hw = nc.sync  # HWDGE dma engine

    # --- constants -----------------------------------------------------------
    id128 = cpool.tile([128, 128], bf16)
    hw.dma_start(out=id128, in_=d["id128"])
    ones_row = cpool.tile([1, S], bf16)
    hw.dma_start(out=ones_row, in_=d["ones_row"])
    iota_col = cpool.tile([128, 1], f32)
    hw.dma_start(out=iota_col, in_=d["iota_col"])
    ones_col = cpool.tile([128, 1], bf16)
    hw.dma_start(out=ones_col, in_=d["ones_col"])
    valemb = cpool.tile([VV, E], bf16)
    hw.dma_start(out=valemb, in_=d["valemb"])
    ringemb = cpool.tile([VR, E], bf16)
    hw.dma_start(out=ringemb, in_=d["ringemb"])
    genb_pp = cpool.tile([128, NO], f32)
    hw.dma_start(out=genb_pp, in_=d["gen_b_pp"])
    eps_t = cpool.tile([128, 1], f32)
    nc.vector.memset(eps_t, 1e-5)
    # dummy-activation table prefetch: a tiny ACTIVATE with the upcoming
    # function makes the 1.28us ACT_TABLE_LOAD happen while the table is not
    # yet needed (off the LN/FFN critical chains).  The anchor tile supplies
    # a data dependency that pins the op near the intended point in the ACT
    # stream (scale=0 + eps bias makes the result well-defined).
    wrm_out = cpool.tile([1, 1], f32)

    def warm(af, anchor):
        nc.scalar.activation(wrm_out, anchor[0:1, 0:1], af,
                             bias=eps_t[:1, :], scale=0.0)
    lnf_s = cpool.tile([128, 8], f32)
    hw.dma_start(out=lnf_s, in_=d["lnf_s_pp"])
    lnf_b = cpool.tile([128, 8], f32)
    hw.dma_start(out=lnf_b, in_=d["lnf_b_pp"])

    # --- embedding -----------------------------------------------------------
    with nc.named_scope("embed"):
        valR = tmppool.tile([VV, S], bf16, tag="sq")
        nc.gpsimd.dma_start(out=valR, in_=d["valrow"].to_broadcast((VV, S)))
        ringR = tmppool.tile([VR, S], bf16, tag="tmp")
        nc.gpsimd.dma_start(out=ringR, in_=d["ringrow"].to_broadcast((VR, S)))
        oh_val = tmppool.tile([VV, S], bf16, tag="sq")
        nc.vector.tensor_scalar(oh_val, valR, iota_col[:VV, :], None, OP.is_equal)
        oh_ring = tmppool.tile([VR, S], bf16, tag="tmp")
        nc.vector.tensor_scalar(oh_ring, ringR, iota_col[:VR, :], None, OP.is_equal)

        h_t = []
        for c in range(NE):
            ps = ps_gemm.tile([128, S], f32, tag="gemm")
            mm(ps, valemb[:, c * 128:(c + 1) * 128], oh_val, start=True, stop=False)
            mm(ps, ringemb[:, c * 128:(c + 1) * 128], oh_ring, start=False, stop=True)
            ht = hpool.tile([128, S], bf16, tag="h")
            nc.scalar.activation(ht, ps, AF.Copy, scale=float(np.sqrt(E)))
            if "dbg_h0" in d:
                nc.sync.dma_start(out=d["dbg_h0"][c], in_=ht)
            h_t.append(ht)
        warm(AF.Exp, h_t[0])  # prefetch exp table for L0 softmax

    # --- layers --------------------------------------------------------------
    for l in range(L):
        h_t = _layer(nc, tc, d, l, h_t, locals())

    # --- final LN + head (lnf scale/bias folded into genW/gen_b) -------------
    with nc.named_scope("final"):
        hf = _layernorm(nc, h_t, None, None, ones_col, ones_row, eps_t,
                        ps_gemm, smallf, smallb, tmppool, hpool, recpool)
        genw_sb = []
        for g in range(2):
            wt = wpool.tile([128, 8, 640], bf16, tag="w")
            hw.dma_start(out=wt, in_=d["genw"][g])
            genw_sb.append(wt)
        for mt in range(NO):
            g, mi = divmod(mt, 5)
            ps = ps_gemm.tile([128, S], f32, tag="gemm")
            for c in range(NE):
                mm(ps, genw_sb[g][:, c, mi * 128:(mi + 1) * 128], hf[c],
                   start=(c == 0), stop=(c == NE - 1))
            ot = outpool.tile([128, S], f32, tag="f32out")
            nc.scalar.activation(ot, ps, AF.Identity, bias=genb_pp[:, mt:mt + 1])
            hw.dma_start(out=d["logits"][mt], in_=ot)


def _layernorm(nc, r_t, s_pp, b_pp, ones_col, ones_row, eps_t,
               ps_gemm, smallf, smallb, tmppool, hpool, recpool):
    """r_t: 8 bf16 [128, S] feature-major tiles -> returns normalized tiles.

    When s_pp is None the affine (scale/bias) is NOT applied: the returned
    tiles are raw (x-mean)*rstd; callers consume them through weights with
    the scale folded in (and re-apply the scale on the residual path)."""
    mm = nc.tensor.matmul
    sums_r = ps_gemm.tile([1, S], f32, tag="gemm", name="lnsum_r")
    sums_q = ps_gemm.tile([1, S], f32, tag="gemm", name="lnsum_q")
    sq_t = []
    for c in range(NE):
        sq = tmppool.tile([128, S], bf16, tag="sq")
        nc.vector.tensor_mul(sq, r_t[c], r_t[c])
        sq_t.append(sq)
    for c in range(NE):
        mm(sums_r, ones_col, r_t[c], start=(c == 0), stop=(c == NE - 1))
    for c in range(NE):
        mm(sums_q, ones_col, sq_t[c], start=(c == 0), stop=(c == NE - 1))

    s2 = smallf.tile([1, S], f32, tag="sf")
    nc.scalar.activation(s2, sums_r, AF.Square)
    varE = smallf.tile([1, S], f32, tag="sf")
    # varE = sumsq - s2/E  (= E * var)
    nc.vector.scalar_tensor_tensor(varE, s2, -1.0 / E, sums_q,
                                   OP.mult, OP.add)
    std = smallf.tile([1, S], f32, tag="sf")
    nc.scalar.activation(std, varE, AF.Sqrt, bias=eps_t[:1, :], scale=1.0 / E)
    rstd = smallf.tile([1, S], f32, tag="sf")
    nc.vector.reciprocal_approx_fast(out=rstd, in_=std)
    ru_b = smallb.tile([1, 2 * S], bf16, tag="sb")
    nc.vector.tensor_copy(ru_b[:, 0:S], rstd)
    # u = mean * rstd = (sum/E) * rstd
    nc.vector.scalar_tensor_tensor(ru_b[:, S:2 * S], sums_r, 1.0 / E,
                                   rstd, OP.mult, OP.mult)
    # broadcast both rows across partitions on the PE (ones ⊗ row)
    rstdR = ps_gemm.tile([128, S], f32, tag="gemm", name="lnrbc")
    mm(rstdR, ones_row[:, 0:128], ru_b[:, 0:S], start=True, stop=True)
    uR = ps_gemm.tile([128, S], f32, tag="gemm", name="lnubc")
    mm(uR, ones_row[:, 0:128], ru_b[:, S:2 * S], start=True, stop=True)

    out_t = []
    for c in range(NE):
        t1 = tmppool.tile([128, S], bf16, tag="tmp")
        nc.vector.tensor_mul(t1, r_t[c], rstdR)
        if s_pp is None:
            ht = hpool.tile([128, S], bf16, tag="h")
            nc.vector.tensor_sub(ht, t1, uR)
        else:
            t2 = tmppool.tile([128, S], bf16, tag="tmp")
            nc.vector.tensor_sub(t2, t1, uR)
            ht = hpool.tile([128, S], bf16, tag="h")
            nc.scalar.activation(ht, t2, AF.Identity,
                                 bias=b_pp[:, c:c + 1], scale=s_pp[:, c:c + 1])
        out_t.append(ht)
    return out_t


def _layer(nc, tc, d, l, h_t, env):
    mm = nc.tensor.matmul
    hw = nc.sync
    wpool = env["wpool"]; hpool = env["hpool"]; qkpool = env["qkpool"]
    vpool = env["vpool"]; maskpool = env["maskpool"]; atpool = env["atpool"]
    ctxpool = env["ctxpool"]; ffpool = env["ffpool"]; tmppool = env["tmppool"]
    smallf = env["smallf"]; smallb = env["smallb"]; recpool = env["recpool"]
    pppool = env["pppool"]
    ps_gemm = env["ps_gemm"]; ps_ctx = env["ps_ctx"]
    ones_row = env["ones_row"]; ones_col = env["ones_col"]; id128 = env["id128"]
    warm = env["warm"]

    # per-layer small params
    bqkv_pp = pppool.tile([128, 16], f32, tag="pp16")
    hw.dma_start(out=bqkv_pp, in_=d["bqkv_pp"][l])
    bo_pp = pppool.tile([128, 8], f32, tag="pp8")
    hw.dma_start(out=bo_pp, in_=d["bo_pp"][l])
    b1_pp = pppool.tile([128, 32], f32, tag="pp32")
    hw.dma_start(out=b1_pp, in_=d["b1_pp"][l])
    b2_pp = pppool.tile([128, 8], f32, tag="pp8")
    hw.dma_start(out=b2_pp, in_=d["b2_pp"][l])
    # ln1 scale (for the r2 residual STT) and, for l>=1, the previous
    # layer's ln2 scale (for the r1 residual STT)
    s1_pp = pppool.tile([128, 8], f32, tag="pp8", name=f"lns1_{l}")
    hw.dma_start(out=s1_pp, in_=d["ln_s_pp"][l, 0])
    if l >= 1:
        s2p_pp = pppool.tile([128, 8], f32, tag="pp8", name=f"lns2p_{l}")
        hw.dma_start(out=s2p_pp, in_=d["ln_s_pp"][l - 1, 1])
    else:
        s2p_pp = None
    if l == L - 1:
        # last layer's ln2 is applied in full (the final LN re-normalizes it)
        ln2_s = pppool.tile([128, 8], f32, tag="pp8", name=f"lns2_{l}")
        ln2_b = pppool.tile([128, 8], f32, tag="pp8", name=f"lnb2_{l}")
        hw.dma_start(out=ln2_s, in_=d["ln_s_pp"][l, 1])
        hw.dma_start(out=ln2_b, in_=d["ln_b_pp"][l, 1])

    # --- QKV -----------------------------------------------------------------
    with nc.named_scope(f"L{l}_qkv"):
        qk_t = []  # 16 tiles: q 0..7, k 8..15
        for g in range(4):  # Q, K feature-major
            wt = wpool.tile([128, 8, 512], bf16, tag="w")
            hw.dma_start(out=wt, in_=d["wqkv"][l, g])
            for mi in range(4):
                mt = g * 4 + mi
                ps = ps_gemm.tile([128, S], f32, tag="gemm")
                for c in range(NE):
                    mm(ps, wt[:, c, mi * 128:(mi + 1) * 128], h_t[c],
                       start=(c == 0), stop=(c == NE - 1))
                qk = qkpool.tile([128, S], bf16, tag="qk")
                nc.scalar.activation(qk, ps, AF.Identity,
                                     bias=bqkv_pp[:, mt:mt + 1])
                if l == 0 and "dbg_qk" in d:
                    hw.dma_start(out=d["dbg_qk"][mt], in_=qk)
                qk_t.append(qk)
        # V token-major, augmented with ones column
        v_t = []
        for n in range(4):
            vt = vpool.tile([128, H, DH + 1], bf16, tag="v")
            nc.vector.memset(vt[:, :, DH:DH + 1], 1.0)
            v_t.append(vt)
        for g in range(2):
            wt = wpool.tile([128, 8, 512], bf16, tag="w")
            hw.dma_start(out=wt, in_=d["wqkv"][l, 4 + g])
            for n in range(4):
                ps = ps_gemm.tile([128, S], f32, tag="gemm")
                for c in range(NE):
                    mm(ps, h_t[c][:, n * 128:(n + 1) * 128], wt[:, c, :],
                       start=(c == 0), stop=(c == NE - 1))
                nc.scalar.activation(
                    v_t[n][:, g * 8:(g + 1) * 8, 0:DH],
                    ps.rearrange("p (a b) -> p a b", a=8), AF.Copy)

    if l == 0 and "dbg_v" in d:
        for n in range(4):
            hw.dma_start(out=d["dbg_v"][n], in_=v_t[n])

    # --- attention ------------------------------------------------------------
    # Triangular: chunk kc (keys kc*128..kc*128+127) only computes queries
    # q >= kc*128 (width MW[kc]); the additive bias becomes a multiplicative
    # exp-mask applied on DVE after the exp.
    with nc.named_scope(f"L{l}_attn"):
        ctx_p = [ctxpool.tile([128, S], bf16, tag="ctx", name=f"cp{l}_{j}")
                 for j in range(8)]
        at_q = {}

        def emit_scores(h):
            mask_t = maskpool.tile([128, MTOT], bf16, tag="mask", name=f"mk{l}_{h}")
            hw.dma_start(out=mask_t, in_=d["mask"][h])
            qt = qk_t[h // 2]
            kt = qk_t[8 + h // 2]
            r0 = (h % 2) * DH
            ate = atpool.tile([128, MTOT], bf16, tag="ate", bufs=3,
                              name=f"e{l}_{h}")
            for kc in range(2):
                w = MW[kc]
                q0 = kc * 128
                sps = ps_gemm.tile([128, S], f32, tag="gemm", name=f"s{l}_{h}_{kc}")
                mm(sps[:, 0:w], kt[r0:r0 + DH, q0:q0 + 128],
                   qt[r0:r0 + DH, q0:S], start=True, stop=True)
                nc.scalar.activation(ate[:, MOFF[kc]:MOFF[kc] + w],
                                     sps[:, 0:w], AF.Exp)
            # kc=2 and kc=3 share one PSUM tile (384 cols) and one exp
            sps23 = ps_gemm.tile([128, S], f32, tag="gemm", name=f"s{l}_{h}_23")
            mm(sps23[:, 0:256], kt[r0:r0 + DH, 256:384],
               qt[r0:r0 + DH, 256:S], start=True, stop=False)
            mm(sps23[:, 256:384], kt[r0:r0 + DH, 384:512],
               qt[r0:r0 + DH, 384:S], start=False, stop=True)
            nc.scalar.activation(ate[:, MOFF[2]:MOFF[2] + 384],
                                 sps23[:, 0:384], AF.Exp)
            # one dense multiply applies the exp-mask for all 4 chunks
            at = atpool.tile([128, MTOT], bf16, tag="at", bufs=4,
                             name=f"a{l}_{h}")
            nc.vector.tensor_mul(at, ate, mask_t)
            at_q[h] = at

        def emit_av(h):
            cps = ps_ctx.tile([DH + 1, S], f32, tag="ctxps", name=f"c{l}_{h}")
            at = at_q.pop(h)
            for kc in range(4):
                w = MW[kc]
                q0 = kc * 128
                mm(cps[:, q0:S], v_t[kc][:, h, :],
                   at[:, MOFF[kc]:MOFF[kc] + w],
                   start=(kc == 0), stop=(kc == 3))
            srow = smallf.tile([1, S], f32, tag="sf", name=f"sr{l}_{h}")
            nc.vector.tensor_copy(srow, cps[DH:DH + 1, :])
            rec = smallf.tile([1, S], f32, tag="sf", name=f"re{l}_{h}")
            nc.vector.reciprocal_approx_fast(out=rec, in_=srow)
            recR = recpool.tile([DH, S], f32, tag="rec", name=f"rr{l}_{h}")
            nc.gpsimd.partition_broadcast(recR, rec, channels=DH)
            p0 = (h % 2) * DH
            nc.vector.tensor_mul(ctx_p[h // 2][p0:p0 + DH, :],
                                 cps[0:DH, :], recR)

        emit_scores(0)
        for h in range(1, H):
            emit_scores(h)
            emit_av(h - 1)
        emit_av(H - 1)
        # prefetch sqrt table for LN1 during out-proj (anchored after the
        # last head's softmax so it cannot evict the exp table early)
        warm(AF.Sqrt, ctx_p[7][DH:DH + 1, :])

        # out-proj (K=128: 2 heads per packed ctx tile) + residual.
        # j-major in two passes of 4 output tiles: PE can start on early
        # ctx tiles while the last heads' softmax chains are still draining.
        wts = []
        for og in range(4):
            wt = wpool.tile([128, 8, 256], bf16, tag="wo", bufs=4,
                            name=f"wo{l}_{og}")
            hw.dma_start(out=wt, in_=d["wo"][l, og])
            wts.append(wt)
        r1_t = []
        for half in range(2):
            mts = [half * 4 + i for i in range(4)]
            pss = [ps_gemm.tile([128, S], f32, tag="gemm",
                                name=f"op{l}_{mt}") for mt in mts]
            for j in range(8):
                for i, mt in enumerate(mts):
                    og, mi = divmod(mt, 2)
                    mm(pss[i], wts[og][:, j, mi * 128:(mi + 1) * 128],
                       ctx_p[j], start=(j == 0), stop=(j == 7))
            for i, mt in enumerate(mts):
                rtmp = tmppool.tile([128, S], bf16, tag="tmp")
                nc.scalar.activation(rtmp, pss[i], AF.Identity,
                                     bias=bo_pp[:, mt:mt + 1])
                r1 = hpool.tile([128, S], bf16, tag="h")
                if s2p_pp is None:
                    nc.vector.tensor_add(r1, rtmp, h_t[mt])
                else:
                    # r1 = h_raw * ln2_s(prev) + (attn_out+bo+ln2_b(prev))
                    nc.vector.scalar_tensor_tensor(
                        r1, h_t[mt], s2p_pp[:, mt:mt + 1], rtmp,
                        OP.mult, OP.add)
                if l == 0 and "dbg_r1" in d:
                    hw.dma_start(out=d["dbg_r1"][mt], in_=r1)
                r1_t.append(r1)

    with nc.named_scope(f"L{l}_ln1"):
        h1_t = _layernorm(nc, r1_t, None, None, ones_col, ones_row,
                          env["eps_t"], ps_gemm, smallf, smallb,
                          tmppool, hpool, recpool)
        warm(AF.Gelu, h1_t[0])  # prefetch gelu table for FFN1
        if l == 0 and "dbg_h1" in d:
            for c in range(NE):
                hw.dma_start(out=d["dbg_h1"][c], in_=h1_t[c])

    # --- FFN -----------------------------------------------------------------
    with nc.named_scope(f"L{l}_ffn"):
        ff_t = []
        for g in range(8):
            wt = wpool.tile([128, 8, 512], bf16, tag="w")
            hw.dma_start(out=wt, in_=d["w1"][l, g])
            for mi in range(4):
                mt = g * 4 + mi
                ps = ps_gemm.tile([128, S], f32, tag="gemm")
                for c in range(NE):
                    mm(ps, wt[:, c, mi * 128:(mi + 1) * 128], h1_t[c],
                       start=(c == 0), stop=(c == NE - 1))
                ft = ffpool.tile([128, S], bf16, tag="ff")
                nc.scalar.activation(ft, ps, AF.Gelu,
                                     bias=b1_pp[:, mt:mt + 1])
                ff_t.append(ft)
        warm(AF.Sqrt, ff_t[31])  # prefetch sqrt table for LN2 during FFN2
        r2_t = [None] * NE
        for half in range(2):
            pss = [ps_gemm.tile([128, S], f32, tag="gemm",
                                name=f"ff2ps{l}_{half}_{i}") for i in range(4)]
            for cg in range(4):
                wt = wpool.tile([128, 8, 512], bf16, tag="w")
                hw.dma_start(out=wt, in_=d["w2"][l, half, cg])
                for c8 in range(8):
                    c = cg * 8 + c8
                    for mi in range(4):
                        mm(pss[mi], wt[:, c8, mi * 128:(mi + 1) * 128], ff_t[c],
                           start=(c == 0), stop=(c == 31))
            for mi in range(4):
                mt = half * 4 + mi
                rtmp = tmppool.tile([128, S], bf16, tag="tmp")
                nc.scalar.activation(rtmp, pss[mi], AF.Identity,
                                     bias=b2_pp[:, mt:mt + 1])
                r2 = hpool.tile([128, S], bf16, tag="h")
                # r2 = h1_raw * ln1_s + (ff_out + b2 + ln1_b)
                nc.vector.scalar_tensor_tensor(
                    r2, h1_t[mt], s1_pp[:, mt:mt + 1], rtmp,
                    OP.mult, OP.add)
                r2_t[mt] = r2

    with nc.named_scope(f"L{l}_ln2"):
        if l == L - 1:
            h2_t = _layernorm(nc, r2_t, ln2_s, ln2_b, ones_col, ones_row,
                              env["eps_t"], ps_gemm, smallf, smallb,
                              tmppool, hpool, recpool)
        else:
            h2_t = _layernorm(nc, r2_t, None, None, ones_col, ones_row,
                              env["eps_t"], ps_gemm, smallf, smallb,
                              tmppool, hpool, recpool)
        if l < L - 1:
            warm(AF.Exp, h2_t[0])  # prefetch exp table for the next softmax
        if l == 0 and "dbg_h2" in d:
            for c in range(NE):
                hw.dma_start(out=d["dbg_h2"][c], in_=h2_t[c])
    return h2_t


def _build():
    if "nc" in _CACHE:
        return _CACHE["nc"]
    from contextlib import ExitStack

    nc = bacc.Bacc("TRN2", debug=False)
    d = _declare(nc)
    with tile.TileContext(nc) as tc:
        with ExitStack() as ctx:
            _emit(nc, tc, d, ctx)
    nc.compile()
    _CACHE["nc"] = nc
    return nc


def kernel_internal(inputs, trace=False, trace_kwargs=None):
    shared = _prep_shared(inputs)
    cores = _prep_percore(inputs)
    nc = _build()
    in_maps = []
    for b in range(B):
        m = dict(shared)
        m.update(cores[b])
        in_maps.append(m)
    res = run_bass_kernel_spmd(
        nc, in_maps, core_ids=list(range(B)), trace=trace,
        **(trace_kwargs or {}),
    )
    outs = []
    for b in range(B):
        lo = res.results[b]["logits"]  # [10, 128, 512]
        lo = lo.reshape(NO * 128, S)[:VV * VR].T  # [512, 1200]
        outs.append(lo)
    out = np.stack(outs).astype(np.float32)  # [B, S, 1200]
    return out, res


def kernel(**inputs):
    out, _ = kernel_internal(inputs)
    return out



# revision 51
# speedup vs baseline: 1.1952x; 1.1952x over previous
"""Trainium2 Bass kernel for nn_BaseGenerator (4-layer dense transformer).

Strategy: pure data-parallel over batch (B=8 -> 8 NeuronCores, no
collectives).  Each core runs the full transformer on one batch element.
Activations are kept feature-major [E, S] in bf16 so every GEMM contracts
over the partition dim; PSUM accumulates in fp32.

Host-side prep (layout only + the distance-bias embedding gather, which has
no efficient device path at 64B/row):
  - weights transposed/blocked into lhsT layouts, cast to bf16
  - attention additive mask  maskT[h, k, q] = dist_emb[dist[q,k], h]
    (+ -1e9 for causal / key-padding), bf16
  - the 1/sqrt(dh) score scale is folded into Wq/bq
"""

import os
import sys

for _p in ("/opt/trn_rl_repo",):
    if _p not in sys.path:
        sys.path.insert(0, _p)

import ml_dtypes
import numpy as np

import concourse.bass as bass
import concourse.mybir as mybir
import concourse.tile as tile
from concourse import bacc
from concourse.bass_utils import run_bass_kernel_spmd

BF16 = ml_dtypes.bfloat16

L, E, H, F = 4, 1024, 16, 4096
B, S = 8, 512
VV, VR = 40, 30
DIST_V = 200
PAD_ID = 0
DH = E // H  # 64
NE = E // 128  # 8 feature chunks
NO = 10  # logit row tiles (1280 padded)
NEG = -1.0e9

f32 = mybir.dt.float32
bf16 = mybir.dt.bfloat16
AF = mybir.ActivationFunctionType
OP = mybir.AluOpType

_CACHE = {}


# ----------------------------------------------------------------------------
# host-side input prep
# ----------------------------------------------------------------------------

def _prep_shared(inp):
    """Weight-layout prep shared by all cores. Returns dict name->np array."""
    out = {}

    def b16(x):
        return np.ascontiguousarray(x.astype(BF16))

    Wqkv = np.asarray(inp["Wqkv"], np.float32).copy()  # [L, 3E, E]
    bqkv = np.asarray(inp["bqkv"], np.float32).copy()  # [L, 3E]
    W1 = np.asarray(inp["W1"], np.float32).copy()      # [L, F, E]
    b1 = np.asarray(inp["b1"], np.float32).copy()      # [L, F]
    genW = np.asarray(inp["gen_W"], np.float32).copy()  # [1200, E]
    gen_b = np.asarray(inp["gen_b"], np.float32).copy()
    ln1_s = np.asarray(inp["ln1_s"], np.float32)
    ln1_b = np.asarray(inp["ln1_b"], np.float32)
    ln2_s = np.asarray(inp["ln2_s"], np.float32)
    ln2_b = np.asarray(inp["ln2_b"], np.float32)
    lnf_s = np.asarray(inp["lnf_s"], np.float32)
    lnf_b = np.asarray(inp["lnf_b"], np.float32)

    # Fold LN affine (per-feature scale/bias) into the consumers:
    #   h_real = t*s + b  with t the raw (x-mean)*rstd tiles kept on device.
    #   QKV/V of layer l>=1 consume ln2 of layer l-1; FFN1 consumes ln1;
    #   the head consumes the final LN.  Residual adds re-apply s via a
    #   fused scalar_tensor_tensor, with b folded into bo/b2.
    for l in range(1, L):
        bqkv[l] += Wqkv[l] @ ln2_b[l - 1]
        Wqkv[l] = Wqkv[l] * ln2_s[l - 1][None, :]
    for l in range(L):
        b1[l] += W1[l] @ ln1_b[l]
        W1[l] = W1[l] * ln1_s[l][None, :]
    gen_b = gen_b + genW @ lnf_b
    genW = genW * lnf_s[None, :]

    # Fold the V bias through attention into the out-proj bias: softmax
    # weights sum to 1, so a V bias adds the constant bv to every ctx vector
    # -> bo += Wo @ bv.  The V GEMM then needs no bias pass at all.
    Wo_raw = np.asarray(inp["Wo"], np.float32)
    bo = np.asarray(inp["bo"], np.float32).copy()
    for l in range(L):
        bo[l] += Wo_raw[l] @ bqkv[l, 2 * E:]

    # fold attention scale into Q projection (after the LN fold)
    scale = 1.0 / np.sqrt(DH)
    Wqkv[:, :E, :] *= scale
    bqkv[:, :E] *= scale

    def block_lhsT(W, gsize):
        # W: [L?, OUT, IN] -> [.., G, 128, IN//128, gsize] with
        # out[..., g, p, c, o] = W[..., g*gsize + o, c*128 + p]
        *lead, O, I = W.shape
        G = O // gsize
        nc_ = I // 128
        Wb = W.reshape(*lead, G, gsize, nc_, 128)
        Wb = np.moveaxis(Wb, -1, -3)  # [..., G, 128, gsize, nc]
        Wb = np.swapaxes(Wb, -1, -2)  # [..., G, 128, nc, gsize]
        return np.ascontiguousarray(Wb)

    out["wqkv"] = b16(block_lhsT(Wqkv, 512))          # [L, 6, 128, 8, 512]
    # Wo for packed ctx (2 heads per 128-partition tile):
    # wo[l, og, p, j, o] = Wo[l][og*256 + o, j*128 + p]
    Wo = np.asarray(inp["Wo"], np.float32)  # [L, E(out), E(in=ctx)]
    out["wo"] = b16(block_lhsT(Wo, 256))    # [L, 4, 128, 8, 256]
    out["w1"] = b16(block_lhsT(W1, 512))    # [L,8,128,8,512]
    # W2: [L, E, F]; stream tiles [half, cg, 128, 8, 512]:
    W2 = np.asarray(inp["W2"], np.float32)  # out=E, in=F
    w2b = block_lhsT(W2, 512)  # [L, 2, 128, 32, 512]
    w2b = w2b.reshape(L, 2, 128, 4, 8, 512).transpose(0, 1, 3, 2, 4, 5)
    out["w2"] = b16(w2b)  # [L, 2, 4, 128, 8, 512]

    genW_pad = np.zeros((1280, E), np.float32)
    genW_pad[:1200] = genW
    out["genw"] = b16(block_lhsT(genW_pad, 640))  # [2, 128, 8, 640]

    gbp = np.zeros((1280,), np.float32)
    gbp[:1200] = gen_b
    out["gen_b_pp"] = np.ascontiguousarray(gbp.reshape(NO, 128).T)  # [128, 10]

    def pp(v):  # [..., N*128] -> [..., 128, N]
        *lead, N = v.shape
        return np.ascontiguousarray(
            v.reshape(*lead, N // 128, 128).swapaxes(-1, -2).astype(np.float32)
        )

    out["bqkv_pp"] = pp(bqkv[:, : 2 * E])  # [L, 128, 16] (Q scaled)
    # residual-path biases with the producing-LN bias folded in
    bo[1:] += ln2_b[:L - 1]
    out["bo_pp"] = pp(bo)  # [L, 128, 8]
    out["b1_pp"] = pp(b1)  # [L, 128, 32]
    b2 = np.asarray(inp["b2"], np.float32) + ln1_b
    out["b2_pp"] = pp(b2)  # [L, 128, 8]

    ln_s = np.stack([ln1_s, ln2_s], 1)  # [L, 2, E]
    ln_b = np.stack([ln1_b, ln2_b], 1)
    out["ln_s_pp"] = pp(ln_s)  # [L, 2, 128, 8]
    out["ln_b_pp"] = pp(ln_b)
    out["lnf_s_pp"] = pp(lnf_s)  # [128, 8]
    out["lnf_b_pp"] = pp(lnf_b)

    out["valemb"] = b16(np.asarray(inp["val_emb"], np.float32))   # [40, E]
    out["ringemb"] = b16(np.asarray(inp["ring_emb"], np.float32))  # [30, E]

    out["id128"] = b16(np.eye(128, dtype=np.float32))
    out["ones_row"] = b16(np.ones((1, S), np.float32))
    out["iota_col"] = np.ascontiguousarray(np.arange(128, dtype=np.float32).reshape(128, 1))
    out["ones_col"] = b16(np.ones((128, 1), np.float32))
    return out


MW = (512, 384, 256, 128)          # emask widths per kc chunk (q >= kc*128)
MOFF = (0, 512, 896, 1152)         # col offsets of each chunk in the packed mask
MTOT = 1280


def _prep_percore(inp):
    """Per-core tensors: token rows + multiplicative exp-mask (triangular).

    emask[b, h, p, MOFF[kc]+j] = exp(bias) for key k = kc*128+p and query
    q = kc*128+j (0 where masked by causality / key padding)."""
    val = np.asarray(inp["val_sequences"]).astype(np.int64)    # [B, S]
    ring = np.asarray(inp["ring_sequences"]).astype(np.int64)  # [B, S]
    dist = np.asarray(inp["distance_squares"]).astype(np.int64)  # [B, S, S]
    de = np.asarray(inp["dist_emb"], np.float32)  # [200, H]

    # m[b, h, k, q] = exp(de[dist[b, q, k], h]), zeroed where masked
    m = de[dist]                         # [B, S(q), S(k), H]
    m = m.transpose(0, 3, 2, 1)          # [B, H, k, q]
    m = np.exp(m)
    kk = np.arange(S)
    causal = kk[:, None] <= kk[None, :]  # [k, q] keep where k <= q
    m = np.where(causal[None, None], m, 0.0)
    padk = val == PAD_ID  # [B, S]
    m = np.where(padk[:, None, :, None], 0.0, m)
    # pack triangular: chunk kc covers keys [kc*128,(kc+1)*128), q >= kc*128
    em = np.zeros((B, H, 128, MTOT), np.float32)
    for kc in range(4):
        em[:, :, :, MOFF[kc]:MOFF[kc] + MW[kc]] = \
            m[:, :, kc * 128:(kc + 1) * 128, kc * 128:]
    em = em.astype(BF16)

    cores = []
    for b in range(B):
        cores.append({
            "mask": np.ascontiguousarray(em[b]),
            "valrow": np.ascontiguousarray(val[b].reshape(1, S).astype(BF16)),
            "ringrow": np.ascontiguousarray(ring[b].reshape(1, S).astype(BF16)),
        })
    return cores


# ----------------------------------------------------------------------------
# device program
# ----------------------------------------------------------------------------

def _declare(nc):
    d = {}

    def di(name, shape, dt):
        d[name] = nc.dram_tensor(name, list(shape), dt, kind="ExternalInput").ap()

    di("wqkv", (L, 6, 128, 8, 512), bf16)
    di("wo", (L, 4, 128, 8, 256), bf16)
    di("w1", (L, 8, 128, 8, 512), bf16)
    di("w2", (L, 2, 4, 128, 8, 512), bf16)
    di("genw", (2, 128, 8, 640), bf16)
    di("gen_b_pp", (128, NO), f32)
    di("bqkv_pp", (L, 128, 16), f32)
    di("bo_pp", (L, 128, 8), f32)
    di("b1_pp", (L, 128, 32), f32)
    di("b2_pp", (L, 128, 8), f32)
    di("ln_s_pp", (L, 2, 128, 8), f32)
    di("ln_b_pp", (L, 2, 128, 8), f32)
    di("lnf_s_pp", (128, 8), f32)
    di("lnf_b_pp", (128, 8), f32)
    di("valemb", (VV, E), bf16)
    di("ringemb", (VR, E), bf16)
    di("id128", (128, 128), bf16)
    di("ones_row", (1, S), bf16)
    di("iota_col", (128, 1), f32)
    di("ones_col", (128, 1), bf16)
    di("mask", (H, 128, MTOT), bf16)
    di("valrow", (1, S), bf16)
    di("ringrow", (1, S), bf16)
    d["logits"] = nc.dram_tensor(
        "logits", [NO, 128, S], f32, kind="ExternalOutput"
    ).ap()
    if os.environ.get("BG_DEBUG"):
        def do(name, shape):
            d[name] = nc.dram_tensor(name, list(shape), bf16,
                                     kind="ExternalOutput").ap()
        do("dbg_h0", (NE, 128, S))
        do("dbg_qk", (16, 128, S))
        do("dbg_v", (4, 128, H, DH + 1))
        do("dbg_at", (8, 128, S))
        do("dbg_ctx", (H, DH, S))
        do("dbg_r1", (NE, 128, S))
        do("dbg_h1", (NE, 128, S))
        do("dbg_h2", (NE, 128, S))
    return d


def _emit(nc, tc, d, ctx):
    mm = nc.tensor.matmul

    cpool = ctx.enter_context(tc.tile_pool(name="cpool", bufs=1))
    wpool = ctx.enter_context(tc.tile_pool(name="wpool", bufs=3))
    hpool = ctx.enter_context(tc.tile_pool(name="hpool", bufs=17))
    qkpool = ctx.enter_context(tc.tile_pool(name="qkpool", bufs=16))
    vpool = ctx.enter_context(tc.tile_pool(name="vpool", bufs=5))
    maskpool = ctx.enter_context(tc.tile_pool(name="maskpool", bufs=4))
    atpool = ctx.enter_context(tc.tile_pool(name="atpool", bufs=10))
    ctxpool = ctx.enter_context(tc.tile_pool(name="ctxpool", bufs=10))
    ffpool = ctx.enter_context(tc.tile_pool(name="ffpool", bufs=33))
    tmppool = ctx.enter_context(tc.tile_pool(name="tmppool", bufs=4))
    smallf = ctx.enter_context(tc.tile_pool(name="smallf", bufs=4))
    smallb = ctx.enter_context(tc.tile_pool(name="smallb", bufs=6))
    recpool = ctx.enter_context(tc.tile_pool(name="recpool", bufs=2))
    outpool = ctx.enter_context(tc.tile_pool(name="outpool", bufs=2))
    pppool = ctx.enter_context(tc.tile_pool(name="pppool", bufs=4))

    ps_gemm = ctx.enter_context(tc.tile_pool(name="ps_gemm", bufs=4, space="PSUM"))
    ps_ctx = ctx.enter_context(tc.tile_pool(name="ps_ctx", bufs=3, space="PSUM"))
    ps_warm = ctx.enter_context(tc.tile_pool(name="ps_warm", bufs=1, space="PSUM"))

    hw = nc.sync  # HWDGE dma engine

    # --- constants -----------------------------------------------------------
    id128 = cpool.tile([128, 128], bf16)
    hw.dma_start(out=id128, in_=d["id128"])
    ones_row = cpool.tile([1, S], bf16)
    hw.dma_start(out=ones_row, in_=d["ones_row"])
    iota_col = cpool.tile([128, 1], f32)
    hw.dma_start(out=iota_col, in_=d["iota_col"])
    ones_col = cpool.tile([128, 1], bf16)
    hw.dma_start(out=ones_col, in_=d["ones_col"])
    valemb = cpool.tile([VV, E], bf16)
    hw.dma_start(out=valemb, in_=d["valemb"])
    ringemb = cpool.tile([VR, E], bf16)
    hw.dma_start(out=ringemb, in_=d["ringemb"])
    genb_pp = cpool.tile([128, NO], f32)
    hw.dma_start(out=genb_pp, in_=d["gen_b_pp"])
    eps_t = cpool.tile([128, 1], f32)
    nc.vector.memset(eps_t, 1e-5)
    # dummy-activation table prefetch: a tiny ACTIVATE with the upcoming
    # function makes the 1.28us ACT_TABLE_LOAD happen while the table is not
    # yet needed (off the LN/FFN critical chains).  The anchor tile supplies
    # a data dependency that pins the op near the intended point in the ACT
    # stream (scale=0 + eps bias makes the result well-defined).
    wrm_out = cpool.tile([1, 1], f32)

    def warm(af, anchor):
        nc.scalar.activation(wrm_out, anchor[0:1, 0:1], af,
                             bias=eps_t[:1, :], scale=0.0)

    # PE keep-warm: dummy matmuls keep the tensor engine's DVFS p-state high
    # through dependency stalls (the clock drops after ~idle and takes ~3us
    # of continuous work to ramp back).  Anchored on `rhs` so the scheduler
    # places them exactly in the stall window.
    def warm_mm(rhs, n):
        for _ in range(n):
            wp = ps_warm.tile([128, rhs.shape[-1]], f32, tag="wps",
                              name="warmps")
            nc.tensor.matmul(wp, id128, rhs, start=True, stop=True)
    lnf_s = cpool.tile([128, 8], f32)
    hw.dma_start(out=lnf_s, in_=d["lnf_s_pp"])
    lnf_b = cpool.tile([128, 8], f32)
    hw.dma_start(out=lnf_b, in_=d["lnf_b_pp"])

    # --- embedding -----------------------------------------------------------
    with nc.named_scope("embed"):
        valR = tmppool.tile([VV, S], bf16, tag="sq")
        nc.gpsimd.dma_start(out=valR, in_=d["valrow"].to_broadcast((VV, S)))
        ringR = tmppool.tile([VR, S], bf16, tag="tmp")
        nc.gpsimd.dma_start(out=ringR, in_=d["ringrow"].to_broadcast((VR, S)))
        oh_val = tmppool.tile([VV, S], bf16, tag="sq")
        nc.vector.tensor_scalar(oh_val, valR, iota_col[:VV, :], None, OP.is_equal)
        oh_ring = tmppool.tile([VR, S], bf16, tag="tmp")
        nc.vector.tensor_scalar(oh_ring, ringR, iota_col[:VR, :], None, OP.is_equal)

        h_t = []
        for c in range(NE):
            ps = ps_gemm.tile([128, S], f32, tag="gemm")
            mm(ps, valemb[:, c * 128:(c + 1) * 128], oh_val, start=True, stop=False)
            mm(ps, ringemb[:, c * 128:(c + 1) * 128], oh_ring, start=False, stop=True)
            ht = hpool.tile([128, S], bf16, tag="h")
            nc.scalar.activation(ht, ps, AF.Copy, scale=float(np.sqrt(E)))
            if "dbg_h0" in d:
                nc.sync.dma_start(out=d["dbg_h0"][c], in_=ht)
            h_t.append(ht)
        warm(AF.Exp, h_t[0])  # prefetch exp table for L0 softmax

    # --- layers --------------------------------------------------------------
    for l in range(L):
        h_t = _layer(nc, tc, d, l, h_t, locals())

    # --- final LN + head (lnf scale/bias folded into genW/gen_b) -------------
    with nc.named_scope("final"):
        hf = _layernorm(nc, h_t, None, None, ones_col, ones_row, eps_t,
                        ps_gemm, smallf, smallb, tmppool, hpool, recpool)
        genw_sb = []
        for g in range(2):
            wt = wpool.tile([128, 8, 640], bf16, tag="w")
            hw.dma_start(out=wt, in_=d["genw"][g])
            genw_sb.append(wt)
        for mt in range(NO):
            g, mi = divmod(mt, 5)
            ps = ps_gemm.tile([128, S], f32, tag="gemm")
            for c in range(NE):
                mm(ps, genw_sb[g][:, c, mi * 128:(mi + 1) * 128], hf[c],
                   start=(c == 0), stop=(c == NE - 1))
            ot = outpool.tile([128, S], f32, tag="f32out")
            nc.scalar.activation(ot, ps, AF.Identity, bias=genb_pp[:, mt:mt + 1])
            hw.dma_start(out=d["logits"][mt], in_=ot)


def _layernorm(nc, r_t, s_pp, b_pp, ones_col, ones_row, eps_t,
               ps_gemm, smallf, smallb, tmppool, hpool, recpool):
    """r_t: 8 bf16 [128, S] feature-major tiles -> returns normalized tiles.

    When s_pp is None the affine (scale/bias) is NOT applied: the returned
    tiles are raw (x-mean)*rstd; callers consume them through weights with
    the scale folded in (and re-apply the scale on the residual path)."""
    mm = nc.tensor.matmul
    sums_r = ps_gemm.tile([1, S], f32, tag="gemm", name="lnsum_r")
    sums_q = ps_gemm.tile([1, S], f32, tag="gemm", name="lnsum_q")
    sq_t = []
    for c in range(NE):
        sq = tmppool.tile([128, S], bf16, tag="sq")
        nc.vector.tensor_mul(sq, r_t[c], r_t[c])
        sq_t.append(sq)
    for c in range(NE):
        mm(sums_r, ones_col, r_t[c], start=(c == 0), stop=(c == NE - 1))
    for c in range(NE):
        mm(sums_q, ones_col, sq_t[c], start=(c == 0), stop=(c == NE - 1))

    s2 = smallf.tile([1, S], f32, tag="sf")
    nc.scalar.activation(s2, sums_r, AF.Square)
    varE = smallf.tile([1, S], f32, tag="sf")
    # varE = sumsq - s2/E  (= E * var)
    nc.vector.scalar_tensor_tensor(varE, s2, -1.0 / E, sums_q,
                                   OP.mult, OP.add)
    std = smallf.tile([1, S], f32, tag="sf")
    nc.scalar.activation(std, varE, AF.Sqrt, bias=eps_t[:1, :], scale=1.0 / E)
    rstd = smallf.tile([1, S], f32, tag="sf")
    nc.vector.reciprocal_approx_fast(out=rstd, in_=std)
    ru_b = smallb.tile([1, 2 * S], bf16, tag="sb")
    nc.vector.tensor_copy(ru_b[:, 0:S], rstd)
    # u = mean * rstd = (sum/E) * rstd
    nc.vector.scalar_tensor_tensor(ru_b[:, S:2 * S], sums_r, 1.0 / E,
                                   rstd, OP.mult, OP.mult)
    # broadcast both rows across partitions on the PE (ones ⊗ row)
    rstdR = ps_gemm.tile([128, S], f32, tag="gemm", name="lnrbc")
    mm(rstdR, ones_row[:, 0:128], ru_b[:, 0:S], start=True, stop=True)
    uR = ps_gemm.tile([128, S], f32, tag="gemm", name="lnubc")
    mm(uR, ones_row[:, 0:128], ru_b[:, S:2 * S], start=True, stop=True)

    out_t = []
    for c in range(NE):
        t1 = tmppool.tile([128, S], bf16, tag="tmp")
        nc.vector.tensor_mul(t1, r_t[c], rstdR)
        if s_pp is None:
            ht = hpool.tile([128, S], bf16, tag="h")
            nc.vector.tensor_sub(ht, t1, uR)
        else:
            t2 = tmppool.tile([128, S], bf16, tag="tmp")
            nc.vector.tensor_sub(t2, t1, uR)
            ht = hpool.tile([128, S], bf16, tag="h")
            nc.scalar.activation(ht, t2, AF.Identity,
                                 bias=b_pp[:, c:c + 1], scale=s_pp[:, c:c + 1])
        out_t.append(ht)
    return out_t


def _layer(nc, tc, d, l, h_t, env):
    mm = nc.tensor.matmul
    hw = nc.sync
    wpool = env["wpool"]; hpool = env["hpool"]; qkpool = env["qkpool"]
    vpool = env["vpool"]; maskpool = env["maskpool"]; atpool = env["atpool"]
    ctxpool = env["ctxpool"]; ffpool = env["ffpool"]; tmppool = env["tmppool"]
    smallf = env["smallf"]; smallb = env["smallb"]; recpool = env["recpool"]
    pppool = env["pppool"]
    ps_gemm = env["ps_gemm"]; ps_ctx = env["ps_ctx"]
    ones_row = env["ones_row"]; ones_col = env["ones_col"]; id128 = env["id128"]
    warm = env["warm"]

    # per-layer small params
    bqkv_pp = pppool.tile([128, 16], f32, tag="pp16")
    hw.dma_start(out=bqkv_pp, in_=d["bqkv_pp"][l])
    bo_pp = pppool.tile([128, 8], f32, tag="pp8")
    hw.dma_start(out=bo_pp, in_=d["bo_pp"][l])
    b1_pp = pppool.tile([128, 32], f32, tag="pp32")
    hw.dma_start(out=b1_pp, in_=d["b1_pp"][l])
    b2_pp = pppool.tile([128, 8], f32, tag="pp8")
    hw.dma_start(out=b2_pp, in_=d["b2_pp"][l])
    # ln1 scale (for the r2 residual STT) and, for l>=1, the previous
    # layer's ln2 scale (for the r1 residual STT)
    s1_pp = pppool.tile([128, 8], f32, tag="pp8", name=f"lns1_{l}")
    hw.dma_start(out=s1_pp, in_=d["ln_s_pp"][l, 0])
    if l >= 1:
        s2p_pp = pppool.tile([128, 8], f32, tag="pp8", name=f"lns2p_{l}")
        hw.dma_start(out=s2p_pp, in_=d["ln_s_pp"][l - 1, 1])
    else:
        s2p_pp = None
    if l == L - 1:
        # last layer's ln2 is applied in full (the final LN re-normalizes it)
        ln2_s = pppool.tile([128, 8], f32, tag="pp8", name=f"lns2_{l}")
        ln2_b = pppool.tile([128, 8], f32, tag="pp8", name=f"lnb2_{l}")
        hw.dma_start(out=ln2_s, in_=d["ln_s_pp"][l, 1])
        hw.dma_start(out=ln2_b, in_=d["ln_b_pp"][l, 1])

    # --- QKV -----------------------------------------------------------------
    with nc.named_scope(f"L{l}_qkv"):
        qk_t = []  # 16 tiles: q 0..7, k 8..15
        for g in range(4):  # Q, K feature-major
            wt = wpool.tile([128, 8, 512], bf16, tag="w")
            hw.dma_start(out=wt, in_=d["wqkv"][l, g])
            for mi in range(4):
                mt = g * 4 + mi
                ps = ps_gemm.tile([128, S], f32, tag="gemm")
                for c in range(NE):
                    mm(ps, wt[:, c, mi * 128:(mi + 1) * 128], h_t[c],
                       start=(c == 0), stop=(c == NE - 1))
                qk = qkpool.tile([128, S], bf16, tag="qk")
                nc.scalar.activation(qk, ps, AF.Identity,
                                     bias=bqkv_pp[:, mt:mt + 1])
                if l == 0 and "dbg_qk" in d:
                    hw.dma_start(out=d["dbg_qk"][mt], in_=qk)
                qk_t.append(qk)
        # V token-major, augmented with ones column
        v_t = []
        for n in range(4):
            vt = vpool.tile([128, H, DH + 1], bf16, tag="v")
            nc.vector.memset(vt[:, :, DH:DH + 1], 1.0)
            v_t.append(vt)
        for g in range(2):
            wt = wpool.tile([128, 8, 512], bf16, tag="w")
            hw.dma_start(out=wt, in_=d["wqkv"][l, 4 + g])
            for n in range(4):
                ps = ps_gemm.tile([128, S], f32, tag="gemm")
                for c in range(NE):
                    mm(ps, h_t[c][:, n * 128:(n + 1) * 128], wt[:, c, :],
                       start=(c == 0), stop=(c == NE - 1))
                nc.scalar.activation(
                    v_t[n][:, g * 8:(g + 1) * 8, 0:DH],
                    ps.rearrange("p (a b) -> p a b", a=8), AF.Copy)

    if l == 0 and "dbg_v" in d:
        for n in range(4):
            hw.dma_start(out=d["dbg_v"][n], in_=v_t[n])

    # --- attention ------------------------------------------------------------
    # Triangular: chunk kc (keys kc*128..kc*128+127) only computes queries
    # q >= kc*128 (width MW[kc]); the additive bias becomes a multiplicative
    # exp-mask applied on DVE after the exp.
    with nc.named_scope(f"L{l}_attn"):
        ctx_p = [ctxpool.tile([128, S], bf16, tag="ctx", name=f"cp{l}_{j}")
                 for j in range(8)]
        at_q = {}

        def emit_scores(h):
            mask_t = maskpool.tile([128, MTOT], bf16, tag="mask", name=f"mk{l}_{h}")
            hw.dma_start(out=mask_t, in_=d["mask"][h])
            qt = qk_t[h // 2]
            kt = qk_t[8 + h // 2]
            r0 = (h % 2) * DH
            ate = atpool.tile([128, MTOT], bf16, tag="ate", bufs=3,
                              name=f"e{l}_{h}")
            for kc in range(2):
                w = MW[kc]
                q0 = kc * 128
                sps = ps_gemm.tile([128, S], f32, tag="gemm", name=f"s{l}_{h}_{kc}")
                mm(sps[:, 0:w], kt[r0:r0 + DH, q0:q0 + 128],
                   qt[r0:r0 + DH, q0:S], start=True, stop=True)
                nc.scalar.activation(ate[:, MOFF[kc]:MOFF[kc] + w],
                                     sps[:, 0:w], AF.Exp)
            # kc=2 and kc=3 share one PSUM tile (384 cols) and one exp
            sps23 = ps_gemm.tile([128, S], f32, tag="gemm", name=f"s{l}_{h}_23")
            mm(sps23[:, 0:256], kt[r0:r0 + DH, 256:384],
               qt[r0:r0 + DH, 256:S], start=True, stop=False)
            mm(sps23[:, 256:384], kt[r0:r0 + DH, 384:512],
               qt[r0:r0 + DH, 384:S], start=False, stop=True)
            nc.scalar.activation(ate[:, MOFF[2]:MOFF[2] + 384],
                                 sps23[:, 0:384], AF.Exp)
            # one dense multiply applies the exp-mask for all 4 chunks
            at = atpool.tile([128, MTOT], bf16, tag="at", bufs=4,
                             name=f"a{l}_{h}")
            nc.vector.tensor_mul(at, ate, mask_t)
            at_q[h] = at

        def emit_av(h):
            cps = ps_ctx.tile([DH + 1, S], f32, tag="ctxps", name=f"c{l}_{h}")
            at = at_q.pop(h)
            for kc in range(4):
                w = MW[kc]
                q0 = kc * 128
                mm(cps[:, q0:S], v_t[kc][:, h, :],
                   at[:, MOFF[kc]:MOFF[kc] + w],
                   start=(kc == 0), stop=(kc == 3))
            srow = smallf.tile([1, S], f32, tag="sf", name=f"sr{l}_{h}")
            nc.vector.tensor_copy(srow, cps[DH:DH + 1, :])
            rec = smallf.tile([1, S], f32, tag="sf", name=f"re{l}_{h}")
            nc.vector.reciprocal_approx_fast(out=rec, in_=srow)
            recR = recpool.tile([DH, S], f32, tag="rec", name=f"rr{l}_{h}")
            nc.gpsimd.partition_broadcast(recR, rec, channels=DH)
            p0 = (h % 2) * DH
            nc.vector.tensor_mul(ctx_p[h // 2][p0:p0 + DH, :],
                                 cps[0:DH, :], recR)

        emit_scores(0)
        for h in range(1, H):
            emit_scores(h)
            emit_av(h - 1)
        emit_av(H - 1)
        # prefetch sqrt table for LN1 during out-proj (anchored after the
        # last head's softmax so it cannot evict the exp table early)
        warm(AF.Sqrt, ctx_p[7][DH:DH + 1, :])

        # out-proj (K=128: 2 heads per packed ctx tile) + residual.
        # j-major in two passes of 4 output tiles: PE can start on early
        # ctx tiles while the last heads' softmax chains are still draining.
        wts = []
        for og in range(4):
            wt = wpool.tile([128, 8, 256], bf16, tag="wo", bufs=4,
                            name=f"wo{l}_{og}")
            hw.dma_start(out=wt, in_=d["wo"][l, og])
            wts.append(wt)
        r1_t = []
        for half in range(2):
            mts = [half * 4 + i for i in range(4)]
            pss = [ps_gemm.tile([128, S], f32, tag="gemm",
                                name=f"op{l}_{mt}") for mt in mts]
            for j in range(8):
                for i, mt in enumerate(mts):
                    og, mi = divmod(mt, 2)
                    mm(pss[i], wts[og][:, j, mi * 128:(mi + 1) * 128],
                       ctx_p[j], start=(j == 0), stop=(j == 7))
            for i, mt in enumerate(mts):
                rtmp = tmppool.tile([128, S], bf16, tag="tmp")
                nc.scalar.activation(rtmp, pss[i], AF.Identity,
                                     bias=bo_pp[:, mt:mt + 1])
                r1 = hpool.tile([128, S], bf16, tag="h")
                if s2p_pp is None:
                    nc.vector.tensor_add(r1, rtmp, h_t[mt])
                else:
                    # r1 = h_raw * ln2_s(prev) + (attn_out+bo+ln2_b(prev))
                    nc.vector.scalar_tensor_tensor(
                        r1, h_t[mt], s2p_pp[:, mt:mt + 1], rtmp,
                        OP.mult, OP.add)
                if l == 0 and "dbg_r1" in d:
                    hw.dma_start(out=d["dbg_r1"][mt], in_=r1)
                r1_t.append(r1)

    with nc.named_scope(f"L{l}_ln1"):
        h1_t = _layernorm(nc, r1_t, None, None, ones_col, ones_row,
                          env["eps_t"], ps_gemm, smallf, smallb,
                          tmppool, hpool, recpool)
        warm(AF.Gelu, h1_t[0])  # prefetch gelu table for FFN1
        if l == 0 and "dbg_h1" in d:
            for c in range(NE):
                hw.dma_start(out=d["dbg_h1"][c], in_=h1_t[c])

    # --- FFN -----------------------------------------------------------------
    with nc.named_scope(f"L{l}_ffn"):
        ff_t = []
        for g in range(8):
            wt = wpool.tile([128, 8, 512], bf16, tag="w")
            hw.dma_start(out=wt, in_=d["w1"][l, g])
            for mi in range(4):
                mt = g * 4 + mi
                ps = ps_gemm.tile([128, S], f32, tag="gemm")
                for c in range(NE):
                    mm(ps, wt[:, c, mi * 128:(mi + 1) * 128], h1_t[c],
                       start=(c == 0), stop=(c == NE - 1))
                ft = ffpool.tile([128, S], bf16, tag="ff")
                nc.scalar.activation(ft, ps, AF.Gelu,
                                     bias=b1_pp[:, mt:mt + 1])
                ff_t.append(ft)
        warm(AF.Sqrt, ff_t[31])  # prefetch sqrt table for LN2 during FFN2
        r2_t = [None] * NE
        for half in range(2):
            pss = [ps_gemm.tile([128, S], f32, tag="gemm",
                                name=f"ff2ps{l}_{half}_{i}") for i in range(4)]
            for cg in range(4):
                wt = wpool.tile([128, 8, 512], bf16, tag="w")
                hw.dma_start(out=wt, in_=d["w2"][l, half, cg])
                for c8 in range(8):
                    c = cg * 8 + c8
                    for mi in range(4):
                        mm(pss[mi], wt[:, c8, mi * 128:(mi + 1) * 128], ff_t[c],
                           start=(c == 0), stop=(c == 31))
            for mi in range(4):
                mt = half * 4 + mi
                rtmp = tmppool.tile([128, S], bf16, tag="tmp")
                nc.scalar.activation(rtmp, pss[mi], AF.Identity,
                                     bias=b2_pp[:, mt:mt + 1])
                r2 = hpool.tile([128, S], bf16, tag="h")
                # r2 = h1_raw * ln1_s + (ff_out + b2 + ln1_b)
                nc.vector.scalar_tensor_tensor(
                    r2, h1_t[mt], s1_pp[:, mt:mt + 1], rtmp,
                    OP.mult, OP.add)
                r2_t[mt] = r2

    with nc.named_scope(f"L{l}_ln2"):
        if l == L - 1:
            h2_t = _layernorm(nc, r2_t, ln2_s, ln2_b, ones_col, ones_row,
                              env["eps_t"], ps_gemm, smallf, smallb,
                              tmppool, hpool, recpool)
        else:
            h2_t = _layernorm(nc, r2_t, None, None, ones_col, ones_row,
                              env["eps_t"], ps_gemm, smallf, smallb,
                              tmppool, hpool, recpool)
        if l < L - 1:
            warm(AF.Exp, h2_t[0])  # prefetch exp table for the next softmax
        if l == 0 and "dbg_h2" in d:
            for c in range(NE):
                hw.dma_start(out=d["dbg_h2"][c], in_=h2_t[c])
    return h2_t


def _build():
    if "nc" in _CACHE:
        return _CACHE["nc"]
    from contextlib import ExitStack

    nc = bacc.Bacc("TRN2", debug=False)
    d = _declare(nc)
    with tile.TileContext(nc) as tc:
        with ExitStack() as ctx:
            _emit(nc, tc, d, ctx)
    nc.compile()
    _CACHE["nc"] = nc
    return nc


def kernel_internal(inputs, trace=False, trace_kwargs=None):
    shared = _prep_shared(inputs)
    cores = _prep_percore(inputs)
    nc = _build()
    in_maps = []
    for b in range(B):
        m = dict(shared)
        m.update(cores[b])
        in_maps.append(m)
    res = run_bass_kernel_spmd(
        nc, in_maps, core_ids=list(range(B)), trace=trace,
        **(trace_kwargs or {}),
    )
    outs = []
    for b in range(B):
        lo = res.results[b]["logits"]  # [10, 128, 512]
        lo = lo.reshape(NO * 128, S)[:VV * VR].T  # [512, 1200]
        outs.append(lo)
    out = np.stack(outs).astype(np.float32)  # [B, S, 1200]
    return out, res


def kernel(**inputs):
    out, _ = kernel_internal(inputs)
    return out

